# revision 4
# baseline (speedup 1.0000x reference)
"""
Trainium2 Bass kernel for nn_BertSelfAttention_1580547972513 — v4.

v3 computed one (batch, head-group) config per core on 8 cores with ~10
f32 input arrays per core.  Wall time is dominated by the axon tunnel
(~60 MB/s h2d, ~30 MB/s d2h, ~55 ms latency per device-array transfer),
so v4 minimizes bytes moved and transfer count:

  - ONE NeuronCore, looping the proven per-config program over all 8
    (batch b, head-half g) configs (`NREP=8`).
  - ONE packed bf16 input blob (~14.5 MB): hsT for 4 batches, full
    Wq/Wk/Wv (transposed), the two distance tables (stored once at
    64 partitions, duplicated to 128 on-device), biases, masks.
  - bf16 output (8 MB back instead of 16).
  - projections consume hs/weights directly as bf16 (full PE rate);
    everything downstream of PSUM is unchanged from v3.
"""

import numpy as np
import ml_dtypes

B, S, E, H, D, MAXP = 4, 1024, 1024, 16, 64, 1024
HPC = 8          # heads per config
EOUT = 512       # projection out-cols per config
WIN = 1152
NREP = 8         # (b, g) configs, all on core 0

# ---- blob element offsets (bf16) ----
HS0 = 0                       # [B][E][S]
WQ0 = HS0 + B * E * S         # [E][H*D]
WK0 = WQ0 + E * H * D
WV0 = WK0 + E * H * D
EM0 = WV0 + E * H * D         # [64][2048]
EMR0 = EM0 + 64 * 2048
BQ0 = EMR0 + 64 * 2048        # [H*D]
BK0 = BQ0 + H * D
MK0 = BK0 + H * D             # [B][S]
NBLOB = MK0 + B * S

_CACHE = {}
LAST_RESULTS = None


def _build():
    import concourse.bacc as bacc
    import concourse.bass as bass
    import concourse.mybir as mybir
    import concourse.tile as tile
    from concourse.masks import make_identity
    from contextlib import ExitStack
    import contextlib

    f32 = mybir.dt.float32
    f32r = mybir.dt.float32r
    bf16 = mybir.dt.bfloat16
    wdt = bf16
    AF = mybir.ActivationFunctionType

    nc = bacc.Bacc("TRN2", target_bir_lowering=False, debug=False)

    blob = nc.dram_tensor("blob", [NBLOB], bf16, kind="ExternalInput")
    out_d = nc.dram_tensor("ctx", [NREP * S, EOUT], bf16, kind="ExternalOutput")

    winq = [[nc.dram_tensor(f"winq{r}_{h}", [S * WIN], wdt, kind="Internal")
             for h in range(HPC)] for r in range(NREP)]
    wink = [[nc.dram_tensor(f"wink{r}_{h}", [S * WIN], wdt, kind="Internal")
             for h in range(HPC)] for r in range(NREP)]

    def bap(offset, dims):
        return bass.AP(tensor=blob, offset=offset, ap=dims)

    with tile.TileContext(nc) as tc, ExitStack() as top:
        const = top.enter_context(tc.tile_pool(name="const", bufs=1))

        # ---------------- constants ----------------
        em_sb = const.tile([128, 2048], f32r)
        emr_sb = const.tile([128, 2048], f32r)
        with tc.tile_pool(name="emtmp", bufs=1) as emtmp:
            em_bf = emtmp.tile([128, 2048], bf16)
            nc.sync.dma_start(out=em_bf[0:64, :],
                              in_=bap(EM0, [[2048, 64], [1, 2048]]))
            nc.sync.dma_start(out=em_bf[64:128, :],
                              in_=bap(EM0, [[2048, 64], [1, 2048]]))
            nc.vector.tensor_copy(out=em_sb, in_=em_bf)
            emr_bf = emtmp.tile([128, 2048], bf16)
            nc.sync.dma_start(out=emr_bf[0:64, :],
                              in_=bap(EMR0, [[2048, 64], [1, 2048]]))
            nc.sync.dma_start(out=emr_bf[64:128, :],
                              in_=bap(EMR0, [[2048, 64], [1, 2048]]))
            nc.vector.tensor_copy(out=emr_sb, in_=emr_bf)

        ident_bf = const.tile([128, 128], wdt)
        make_identity(nc, ident_bf)
        ident65 = const.tile([65, 65], f32)
        make_identity(nc, ident65)

        qT_sb = const.tile([128, 4, 1024], f32r)
        kT_sb = const.tile([128, 4, 1024], f32r)
        v_sb = const.tile([128, 8, HPC, 65], f32r)
        # ISA rejects memset on f32r — memset a plain-f32 staging tile and copy
        ones_t = const.tile([128, 64], f32)
        nc.vector.memset(ones_t, 1.0)
        nc.vector.tensor_copy(
            out=v_sb[:, :, :, 64:65],
            in_=ones_t.rearrange("p (a b c) -> p a b c", a=8, b=8))

        ctx_pool = top.enter_context(tc.tile_pool(name="ctxp", bufs=2))
        stage_pool = top.enter_context(tc.tile_pool(name="stage", bufs=3))
        wpsum = top.enter_context(tc.tile_pool(name="wpsum", bufs=2, space="PSUM"))

        def emit_windows(rep, pair):
            """Window matmuls + evictions + DRAM writes for one pair, both
            sides; the two heads' matmuls are interleaved for row-tiling."""
            for side, (src_sb, tab_sb, bufs) in enumerate(
                    ((qT_sb, emr_sb, winq[rep]), (kT_sb, em_sb, wink[rep]))):
                for half in range(2):
                    stages = []
                    for sub in range(2):
                        stages.append(stage_pool.tile(
                            [128, 4, WIN], wdt, tag="stage",
                            name=f"st_{side}_{2 * pair + sub}_{half}"))
                    for li in range(4):
                        lb = half * 4 + li
                        w0 = 896 - 128 * lb
                        pss = [wpsum.tile([128, 3, 512], f32, tag="win",
                                          name=f"w_{side}_{2 * pair + sub}_{lb}")
                               for sub in range(2)]
                        for c in range(3):
                            for sub in range(2):
                                base = 64 * sub
                                nc.tensor.matmul(
                                    pss[sub][:, c, 0:384],
                                    lhsT=src_sb[base:base + 64, pair,
                                                lb * 128:(lb + 1) * 128],
                                    rhs=tab_sb[base:base + 64,
                                               w0 + c * 384: w0 + (c + 1) * 384],
                                    start=True, stop=True)
                        for sub in range(2):
                            dst3 = stages[sub][:, li, :].rearrange(
                                "p (a b) -> p a b", b=384)
                            if (lb + sub) % 2 == 0:
                                nc.vector.tensor_copy(out=dst3,
                                                      in_=pss[sub][:, :, 0:384])
                            else:
                                nc.scalar.activation(out=dst3,
                                                     in_=pss[sub][:, :, 0:384],
                                                     func=AF.Copy)
                    for sub in range(2):
                        h = 2 * pair + sub
                        out_ap = bass.AP(
                            tensor=bufs[h], offset=half * 4 * 128 * WIN,
                            ap=[[WIN, 128], [128 * WIN, 4], [1, WIN]])
                        nc.sync.dma_start(out=out_ap, in_=stages[sub])

        # ---------------- phase 1: projections interleaved with windows ----
        def emit_phase1(rep):
          b, g = divmod(rep, 2)
          with tc.tile_pool(name=f"hs{rep}", bufs=1) as hspool, \
               tc.tile_pool(name=f"ppsum{rep}", bufs=2, space="PSUM") as ppsum:
              bq_bf = hspool.tile([128, 4], bf16)
              nc.scalar.dma_start(out=bq_bf,
                                  in_=bap(BQ0 + g * 512, [[1, 128], [128, 4]]))
              bq_sb = hspool.tile([128, 4], f32)
              nc.vector.tensor_copy(out=bq_sb, in_=bq_bf)
              bk_bf = hspool.tile([128, 4], bf16)
              nc.scalar.dma_start(out=bk_bf,
                                  in_=bap(BK0 + g * 512, [[1, 128], [128, 4]]))
              bk_sb = hspool.tile([128, 4], f32)
              nc.vector.tensor_copy(out=bk_sb, in_=bk_bf)
              mk_bf = hspool.tile([128, 8], bf16)
              nc.scalar.dma_start(out=mk_bf,
                                  in_=bap(MK0 + b * S, [[1, 128], [128, 8]]))
              # lives into phase 2 (exp bias) — allocate from the long-lived
              # double-buffered pool, not this phase-1 pool
              mask_sb = ctx_pool.tile([128, 8], f32, tag="mask",
                                      name=f"mask_{rep}")
              nc.vector.tensor_copy(out=mask_sb, in_=mk_bf)

              hs_sb = hspool.tile([128, 8, 1024], bf16)
              wq_sb = hspool.tile([128, 8, EOUT], bf16)
              wk_sb = hspool.tile([128, 8, EOUT], bf16)
              wv_sb = hspool.tile([128, 8, EOUT], bf16)
              hsr = bap(HS0 + b * E * S, [[S, 128], [128 * S, 8], [1, S]])
              wqr = bap(WQ0 + g * 512, [[1024, 128], [128 * 1024, 8], [1, 512]])
              wkr = bap(WK0 + g * 512, [[1024, 128], [128 * 1024, 8], [1, 512]])
              wvr = bap(WV0 + g * 512, [[1024, 128], [128 * 1024, 8], [1, 512]])
              # interleave so (hs0, wq0) land first
              for cc in range(8):
                  nc.scalar.dma_start(out=hs_sb[:, cc:cc + 1, :],
                                      in_=hsr[:, cc:cc + 1, :])
                  nc.scalar.dma_start(out=wq_sb[:, cc:cc + 1, :],
                                      in_=wqr[:, cc:cc + 1, :])
                  nc.scalar.dma_start(out=wk_sb[:, cc:cc + 1, :],
                                      in_=wkr[:, cc:cc + 1, :])
              nc.scalar.dma_start(out=wv_sb, in_=wvr)

              def proj_qk(w_sb, dst, b_sb, j):
                  for half in range(2):
                      ps = ppsum.tile([128, 512], f32, tag="proj",
                                      name=f"ps_{j}_{half}")
                      for e in range(8):
                          nc.tensor.matmul(
                              ps,
                              lhsT=w_sb[:, e, j * 128:(j + 1) * 128],
                              rhs=hs_sb[:, e, half * 512:(half + 1) * 512],
                              start=(e == 0), stop=(e == 7))
                      nc.vector.tensor_scalar_add(
                          out=dst[:, j, half * 512:(half + 1) * 512], in0=ps,
                          scalar1=b_sb[:, j:j + 1])

              proj_qk(wq_sb, qT_sb, bq_sb, 0)
              proj_qk(wk_sb, kT_sb, bk_sb, 0)
              emit_windows(rep, 0)
              for p in range(1, 4):
                  proj_qk(wq_sb, qT_sb, bq_sb, p)
                  proj_qk(wk_sb, kT_sb, bk_sb, p)

              for t in range(8):
                  psv = ppsum.tile([128, 512], f32, tag="proj", name=f"psv_{t}")
                  for e in range(8):
                      nc.tensor.matmul(
                          psv,
                          lhsT=hs_sb[:, e, t * 128:(t + 1) * 128],
                          rhs=wv_sb[:, e, :],
                          start=(e == 0), stop=(e == 7))
                  nc.vector.tensor_copy(
                      out=v_sb[:, t, :, 0:64],
                      in_=psv.rearrange("p (h d) -> p h d", d=64))
              return mask_sb

        # ---------------- phase 2: scores + pv (pools per rep) ----------
        pools2 = {}

        def open_phase2_pools(rep):
            st = contextlib.ExitStack()
            pools2["skew"] = st.enter_context(
                tc.tile_pool(name=f"skew{rep}", bufs=2))
            pools2["pt"] = st.enter_context(
                tc.tile_pool(name=f"pt{rep}", bufs=8))
            pools2["misc"] = st.enter_context(
                tc.tile_pool(name=f"misc{rep}", bufs=2))
            pools2["spsum"] = st.enter_context(
                tc.tile_pool(name=f"spsum{rep}", bufs=2, space="PSUM"))
            return st

        def emit_skew_reads(rep, h):
            sk = {}
            for rhf in range(2):
                t = pools2["skew"].tile([128, 8, 512], wdt, tag="skq",
                                   name=f"skq_{h}_{rhf}")
                nc.scalar.dma_start(
                    out=t,
                    in_=bass.AP(tensor=winq[rep][h], offset=127 + 512 * rhf,
                                ap=[[WIN - 1, 128], [128 * WIN, 8], [1, 512]]))
                sk[("q", rhf)] = t
            for hf in range(2):
                t = pools2["skew"].tile([128, 4, 1024], wdt, tag="skk",
                                   name=f"skk_{h}_{hf}")
                nc.scalar.dma_start(
                    out=t,
                    in_=bass.AP(tensor=wink[rep][h],
                                offset=127 + hf * 4 * 128 * WIN,
                                ap=[[WIN - 1, 128], [128 * WIN, 4], [1, 1024]]))
                sk[("k", hf)] = t
            return sk

        def emit_scores_pv(rep, pair, sub, sk, mask_sb, ctx_all):
            base = 64 * sub
            h = 2 * pair + sub
            pts = []
            for rb in range(8):
                pt = pools2["pt"].tile([128, 1024], f32r, tag="pt",
                                       name=f"pt_{h}_{rb}")
                for lhalf in range(2):
                    sl = slice(lhalf * 512, (lhalf + 1) * 512)
                    ps_s = pools2["spsum"].tile([128, 512], f32, tag="sc",
                                      name=f"s_{h}_{rb}_{lhalf}")
                    nc.tensor.matmul(
                        ps_s,
                        lhsT=kT_sb[base:base + 64, pair, rb * 128:(rb + 1) * 128],
                        rhs=qT_sb[base:base + 64, pair, sl],
                        start=True, stop=False)
                    nc.tensor.matmul(
                        ps_s,
                        lhsT=ident_bf,
                        rhs=sk[("k", rb // 4)][:, rb % 4, sl],
                        start=False, stop=False)
                    for li in range(4):
                        lc = lhalf * 4 + li
                        nc.tensor.matmul(
                            ps_s[:, li * 128:(li + 1) * 128],
                            lhsT=sk[("q", rb // 4)][:, lc,
                                                    (rb % 4) * 128:(rb % 4 + 1) * 128],
                            rhs=ident_bf,
                            start=False, stop=(li == 3))
                    nc.scalar.activation(out=pt[:, sl], in_=ps_s, func=AF.Exp,
                                         bias=mask_sb[:, rb:rb + 1], scale=0.125)
                pts.append(pt)

            ctxT_ps = wpsum.tile([65, 1024], f32, tag="win", name=f"cT_{h}")
            for rc in range(8):
                for half in range(2):
                    sl = slice(half * 512, (half + 1) * 512)
                    nc.tensor.matmul(
                        ctxT_ps[:, sl],
                        lhsT=v_sb[:, rc, h, :],
                        rhs=pts[rc][:, sl],
                        start=(rc == 0), stop=(rc == 7))
            ctxT_sb = pools2["misc"].tile([65, 1024], f32, tag="ctxT_sb",
                                     name=f"cTs_{h}")
            nc.scalar.activation(out=ctxT_sb, in_=ctxT_ps, func=AF.Copy)

            for lc in range(8):
                ctx_ps = wpsum.tile([128, 65], f32, tag="win", name=f"cp_{h}_{lc}")
                nc.tensor.matmul(
                    ctx_ps,
                    lhsT=ctxT_sb[:, lc * 128:(lc + 1) * 128],
                    rhs=ident65,
                    is_transpose=True)
                recip = pools2["misc"].tile([128, 1], f32, tag="recip",
                                       name=f"rc_{h}_{lc}")
                nc.vector.reciprocal(out=recip, in_=ctx_ps[:, 64:65])
                nc.vector.tensor_scalar_mul(
                    out=ctx_all[:, lc, h * 64:(h + 1) * 64],
                    in0=ctx_ps[:, 0:64],
                    scalar1=recip)

        for rep in range(NREP):
            mask_sb = emit_phase1(rep)
            ctx_all = ctx_pool.tile([128, 8, EOUT], bf16, tag="ctx",
                                    name=f"ctx_{rep}")
            p2 = open_phase2_pools(rep)
            for pair in range(4):
                sk0 = emit_skew_reads(rep, 2 * pair)
                if pair + 1 < 4:
                    emit_windows(rep, pair + 1)
                sk1 = emit_skew_reads(rep, 2 * pair + 1)
                emit_scores_pv(rep, pair, 0, sk0, mask_sb, ctx_all)
                emit_scores_pv(rep, pair, 1, sk1, mask_sb, ctx_all)

            nc.sync.dma_start(
                out=bass.AP(tensor=out_d, offset=rep * S * EOUT,
                            ap=[[EOUT, 128], [128 * EOUT, 8], [1, EOUT]]),
                in_=ctx_all)
            p2.close()

    nc.compile()
    return nc


def get_nc():
    if "nc" not in _CACHE:
        _CACHE["nc"] = _build()
    return _CACHE["nc"]


def make_in_maps(hidden_states, attention_mask, Wq, bq, Wk, bk, Wv, bv, dist_emb):
    bf = ml_dtypes.bfloat16
    f = np.float32
    hidden_states = np.asarray(hidden_states, f)
    dist_emb = np.asarray(dist_emb, f)

    b_ = np.empty((NBLOB,), bf)
    b_[HS0:WQ0] = hidden_states.transpose(0, 2, 1).astype(bf).ravel()
    b_[WQ0:WK0] = np.asarray(Wq, f).T.astype(bf).ravel()
    b_[WK0:WV0] = np.asarray(Wk, f).T.astype(bf).ravel()
    b_[WV0:EM0] = np.asarray(Wv, f).T.astype(bf).ravel()
    emt = np.zeros((64, 2048), f)
    emt[:, :2047] = dist_emb.T
    b_[EM0:EMR0] = emt.astype(bf).ravel()
    emr = np.zeros((64, 2048), f)
    emr[:, :2047] = dist_emb[::-1].T
    b_[EMR0:BQ0] = emr.astype(bf).ravel()
    b_[BQ0:BK0] = np.asarray(bq, f).astype(bf)
    b_[BK0:MK0] = np.asarray(bk, f).astype(bf)
    b_[MK0:NBLOB] = np.asarray(attention_mask, f).reshape(B, S).astype(bf).ravel()
    return [{"blob": b_}]


def assemble(results, bv):
    ctx = np.asarray(results[0]["ctx"], np.float32)  # [NREP*S, EOUT]
    out = np.empty((B, S, E), np.float32)
    for rep in range(NREP):
        b, g = divmod(rep, 2)
        out[b, :, EOUT * g:EOUT * (g + 1)] = ctx[rep * S:(rep + 1) * S]
    out += np.asarray(bv, np.float32)[None, None, :]
    return out


def kernel(hidden_states, attention_mask, Wq, bq, Wk, bk, Wv, bv, dist_emb,
           trace=False):
    global LAST_RESULTS
    from concourse import bass_utils
    nc = get_nc()
    in_maps = make_in_maps(hidden_states, attention_mask, Wq, bq, Wk, bk, Wv,
                           bv, dist_emb)
    res = bass_utils.run_bass_kernel_spmd(nc, in_maps, core_ids=[0],
                                          trace=trace)
    LAST_RESULTS = res
    return assemble(res.results, bv)


# revision 10
# speedup vs baseline: 1.2840x; 1.2840x over previous
"""
Trainium2 Bass kernel for nn_BertSelfAttention_1580547972513 — v5.

Single core, one packed bf16 input blob, bf16 output (v4), plus a
restructure driven by the measured device cost model (~100us per matmul
instruction regardless of shape; DVE ops ~15us; strided/transposing DMA
reads ~free):

  - q-side skew windows are read back from DRAM with a TRANSPOSED strided
    AP directly into [key, query] orientation — removes the 4 identity-
    transpose matmuls per score chunk (512 matmuls/rep).
  - both skews are summed with one DVE tensor_add per head and applied to
    scores with one fused scalar_tensor_tensor (+mask) per chunk, in
    PSUM — removes the per-chunk identity add-matmul (128 matmuls/rep).
  - kT and the q-window table are pre-scaled by 1/8 (host side for the
    table, fused into the k-projection eviction) so no separate score
    scaling is needed.
  - context is shipped unnormalized+transposed as [65,1024] per head
    (64 dims + denominator row); the softmax divide and [d,l]->[l,d]
    transpose happen on the host — removes 64 transpose matmuls and 128
    DVE ops per rep.

matmuls/rep: 192 proj + 384 windows + 128 scores + 128 pv = 832
(v4 had 1536).
"""

import numpy as np
import ml_dtypes

B, S, E, H, D, MAXP = 4, 1024, 1024, 16, 64, 1024
HPC = 8          # heads per config
EOUT = 512       # projection out-cols per config
WIN = 1152
NREP = 8         # (b, g) configs, all on core 0

# ---- blob element offsets (bf16) ----
HS0 = 0                       # [B][E][S]
WQ0 = HS0 + B * E * S         # [E][H*D]
WK0 = WQ0 + E * H * D
WV0 = WK0 + E * H * D
EM0 = WV0 + E * H * D         # [64][2048]
EMR0 = EM0 + 64 * 2048
BQ0 = EMR0 + 64 * 2048        # [H*D]
BK0 = BQ0 + H * D
MK0 = BK0 + H * D             # [B][S]
NBLOB = MK0 + B * S

_CACHE = {}
LAST_RESULTS = None


def _build():
    import concourse.bacc as bacc
    import concourse.bass as bass
    import concourse.mybir as mybir
    import concourse.tile as tile
    from concourse.masks import make_identity
    from contextlib import ExitStack
    import contextlib

    f32 = mybir.dt.float32
    f32r = mybir.dt.float32r
    bf16 = mybir.dt.bfloat16
    wdt = bf16
    AF = mybir.ActivationFunctionType
    OP = mybir.AluOpType

    nc = bacc.Bacc("TRN2", target_bir_lowering=False, debug=False)

    blob = nc.dram_tensor("blob", [NBLOB], bf16, kind="ExternalInput")
    out_d = nc.dram_tensor("ctx", [NREP * HPC * 65 * 1024], bf16,
                           kind="ExternalOutput")

    # q-windows stored UNBANDED [l, 2048] so the transposed skew read
    # merges (lb, p) into one stride-2047 dim (3-dim DMA AP limit).
    WQF = 2048
    winq = [[nc.dram_tensor(f"winq{r}_{h}", [S * WQF], wdt, kind="Internal")
             for h in range(HPC)] for r in range(NREP)]
    wink = [[nc.dram_tensor(f"wink{r}_{h}", [S * WIN], wdt, kind="Internal")
             for h in range(HPC)] for r in range(NREP)]

    def bap(offset, dims):
        return bass.AP(tensor=blob, offset=offset, ap=dims)

    with tile.TileContext(nc) as tc, ExitStack() as top:
        const = top.enter_context(tc.tile_pool(name="const", bufs=1))

        # ---------------- constants ----------------
        em_sb = const.tile([128, 2048], f32r)
        emr_sb = const.tile([128, 2048], f32r)
        with tc.tile_pool(name="emtmp", bufs=1) as emtmp:
            em_bf = emtmp.tile([128, 2048], bf16)
            nc.sync.dma_start(out=em_bf[0:64, :],
                              in_=bap(EM0, [[2048, 64], [1, 2048]]))
            nc.sync.dma_start(out=em_bf[64:128, :],
                              in_=bap(EM0, [[2048, 64], [1, 2048]]))
            nc.vector.tensor_copy(out=em_sb, in_=em_bf)
            emr_bf = emtmp.tile([128, 2048], bf16)
            nc.sync.dma_start(out=emr_bf[0:64, :],
                              in_=bap(EMR0, [[2048, 64], [1, 2048]]))
            nc.sync.dma_start(out=emr_bf[64:128, :],
                              in_=bap(EMR0, [[2048, 64], [1, 2048]]))
            nc.vector.tensor_copy(out=emr_sb, in_=emr_bf)

        qT_sb = const.tile([128, 4, 1024], f32r)
        kT_sb = const.tile([128, 4, 1024], f32r)
        v_sb = const.tile([128, 8, HPC, 65], bf16)
        nc.vector.memset(v_sb[:, :, :, 64:65], 1.0)

        persist = top.enter_context(tc.tile_pool(name="persist", bufs=2))
        stage_pool = top.enter_context(tc.tile_pool(name="stage", bufs=2))
        wpsum = top.enter_context(tc.tile_pool(name="wpsum", bufs=2, space="PSUM"))

        def emit_windows_q(rep, pair):
            """q-side windows, full-width [l, 2048] = qT @ emr (pre-scaled
            /8). Two heads' matmuls interleaved (base partitions 0/64)."""
            bufs = winq[rep]
            for half in range(2):
                stages = [stage_pool.tile([128, 4, WQF], wdt, tag="stq",
                                          name=f"sq_{2 * pair + sub}_{half}")
                          for sub in range(2)]
                for li in range(4):
                    lb = half * 4 + li
                    psA = [wpsum.tile([128, 3, 512], f32, tag="win",
                                      name=f"wqA_{2 * pair + sub}_{lb}")
                           for sub in range(2)]
                    psB = [wpsum.tile([128, 512], f32, tag="win",
                                      name=f"wqB_{2 * pair + sub}_{lb}")
                           for sub in range(2)]
                    for c in range(4):
                        for sub in range(2):
                            base = 64 * sub
                            dst = psA[sub][:, c, :] if c < 3 else psB[sub]
                            nc.tensor.matmul(
                                dst,
                                lhsT=qT_sb[base:base + 64, pair,
                                           lb * 128:(lb + 1) * 128],
                                rhs=emr_sb[base:base + 64,
                                           c * 512:(c + 1) * 512],
                                start=True, stop=True)
                    for sub in range(2):
                        d3 = stages[sub][:, li, 0:1536].rearrange(
                            "p (a b) -> p a b", b=512)
                        if (lb + sub) % 2 == 0:
                            nc.vector.tensor_copy(out=d3, in_=psA[sub])
                            nc.scalar.activation(
                                out=stages[sub][:, li, 1536:2048],
                                in_=psB[sub], func=AF.Copy)
                        else:
                            nc.scalar.activation(out=d3, in_=psA[sub],
                                                 func=AF.Copy)
                            nc.vector.tensor_copy(
                                out=stages[sub][:, li, 1536:2048],
                                in_=psB[sub])
                for sub in range(2):
                    h = 2 * pair + sub
                    out_ap = bass.AP(
                        tensor=bufs[h], offset=half * 4 * 128 * WQF,
                        ap=[[WQF, 128], [128 * WQF, 4], [1, WQF]])
                    nc.sync.dma_start(out=out_ap, in_=stages[sub])

        def emit_windows_k(rep, pair):
            """k-side windows, banded [l, WIN] = (kT/8) @ em."""
            bufs = wink[rep]
            for half in range(2):
                stages = [stage_pool.tile([128, 4, WIN], wdt, tag="stk",
                                          name=f"sk_{2 * pair + sub}_{half}")
                          for sub in range(2)]
                for li in range(4):
                    lb = half * 4 + li
                    w0 = 896 - 128 * lb
                    pss = [wpsum.tile([128, 3, 512], f32, tag="win",
                                      name=f"wk_{2 * pair + sub}_{lb}")
                           for sub in range(2)]
                    for c in range(3):
                        for sub in range(2):
                            base = 64 * sub
                            nc.tensor.matmul(
                                pss[sub][:, c, 0:384],
                                lhsT=kT_sb[base:base + 64, pair,
                                           lb * 128:(lb + 1) * 128],
                                rhs=em_sb[base:base + 64,
                                          w0 + c * 384: w0 + (c + 1) * 384],
                                start=True, stop=True)
                    for sub in range(2):
                        dst3 = stages[sub][:, li, :].rearrange(
                            "p (a b) -> p a b", b=384)
                        if (lb + sub) % 2 == 0:
                            nc.vector.tensor_copy(out=dst3,
                                                  in_=pss[sub][:, :, 0:384])
                        else:
                            nc.scalar.activation(out=dst3,
                                                 in_=pss[sub][:, :, 0:384],
                                                 func=AF.Copy)
                for sub in range(2):
                    h = 2 * pair + sub
                    out_ap = bass.AP(
                        tensor=bufs[h], offset=half * 4 * 128 * WIN,
                        ap=[[WIN, 128], [128 * WIN, 4], [1, WIN]])
                    nc.sync.dma_start(out=out_ap, in_=stages[sub])

        def emit_windows(rep, pair):
            emit_windows_q(rep, pair)
            emit_windows_k(rep, pair)

        # ---------------- phase 1: projections interleaved with windows ----
        def emit_phase1(rep):
          b, g = divmod(rep, 2)
          with tc.tile_pool(name=f"hs{rep}", bufs=1) as hspool, \
               tc.tile_pool(name=f"ppsum{rep}", bufs=2, space="PSUM") as ppsum:
              bq_bf = hspool.tile([128, 4], bf16)
              nc.scalar.dma_start(out=bq_bf,
                                  in_=bap(BQ0 + g * 512, [[1, 128], [128, 4]]))
              bq_sb = hspool.tile([128, 4], f32)
              nc.vector.tensor_copy(out=bq_sb, in_=bq_bf)
              bk_bf = hspool.tile([128, 4], bf16)
              nc.scalar.dma_start(out=bk_bf,
                                  in_=bap(BK0 + g * 512, [[1, 128], [128, 4]]))
              bk_sb = hspool.tile([128, 4], f32)
              nc.vector.tensor_copy(out=bk_sb, in_=bk_bf)
              mk_bf = hspool.tile([128, 8], bf16)
              nc.scalar.dma_start(out=mk_bf,
                                  in_=bap(MK0 + b * S, [[1, 128], [128, 8]]))
              # lives into phase 2 (score bias) — long-lived pool
              mask_sb = persist.tile([128, 8], f32, tag="mask",
                                     name=f"mask_{rep}")
              nc.vector.tensor_copy(out=mask_sb, in_=mk_bf)

              hs_sb = hspool.tile([128, 8, 1024], bf16)
              wq_sb = hspool.tile([128, 8, EOUT], bf16)
              wk_sb = hspool.tile([128, 8, EOUT], bf16)
              wv_sb = hspool.tile([128, 8, EOUT], bf16)
              hsr = bap(HS0 + b * E * S, [[S, 128], [128 * S, 8], [1, S]])
              wqr = bap(WQ0 + g * 512, [[1024, 128], [128 * 1024, 8], [1, 512]])
              wkr = bap(WK0 + g * 512, [[1024, 128], [128 * 1024, 8], [1, 512]])
              wvr = bap(WV0 + g * 512, [[1024, 128], [128 * 1024, 8], [1, 512]])
              # interleave so (hs0, wq0) land first
              for cc in range(8):
                  nc.scalar.dma_start(out=hs_sb[:, cc:cc + 1, :],
                                      in_=hsr[:, cc:cc + 1, :])
                  nc.scalar.dma_start(out=wq_sb[:, cc:cc + 1, :],
                                      in_=wqr[:, cc:cc + 1, :])
                  nc.scalar.dma_start(out=wk_sb[:, cc:cc + 1, :],
                                      in_=wkr[:, cc:cc + 1, :])
              nc.scalar.dma_start(out=wv_sb, in_=wvr)

              def proj_qk(w_sb, dst, b_sb, j, prescale):
                  for half in range(2):
                      ps = ppsum.tile([128, 512], f32, tag="proj",
                                      name=f"ps_{j}_{half}")
                      for e in range(8):
                          nc.tensor.matmul(
                              ps,
                              lhsT=w_sb[:, e, j * 128:(j + 1) * 128],
                              rhs=hs_sb[:, e, half * 512:(half + 1) * 512],
                              start=(e == 0), stop=(e == 7))
                      dst_sl = dst[:, j, half * 512:(half + 1) * 512]
                      if prescale is None:
                          nc.vector.tensor_scalar_add(
                              out=dst_sl, in0=ps, scalar1=b_sb[:, j:j + 1])
                      else:
                          # dst = ps*prescale + bias  (bias pre-scaled on host)
                          nc.vector.tensor_scalar(
                              out=dst_sl, in0=ps, scalar1=prescale,
                              scalar2=b_sb[:, j:j + 1],
                              op0=OP.mult, op1=OP.add)

              proj_qk(wq_sb, qT_sb, bq_sb, 0, None)
              proj_qk(wk_sb, kT_sb, bk_sb, 0, 0.125)
              emit_windows(rep, 0)
              for p in range(1, 4):
                  proj_qk(wq_sb, qT_sb, bq_sb, p, None)
                  proj_qk(wk_sb, kT_sb, bk_sb, p, 0.125)

              for t in range(8):
                  psv = ppsum.tile([128, 512], f32, tag="proj", name=f"psv_{t}")
                  for e in range(8):
                      nc.tensor.matmul(
                          psv,
                          lhsT=hs_sb[:, e, t * 128:(t + 1) * 128],
                          rhs=wv_sb[:, e, :],
                          start=(e == 0), stop=(e == 7))
                  nc.vector.tensor_copy(
                      out=v_sb[:, t, :, 0:64],
                      in_=psv.rearrange("p (h d) -> p h d", d=64))
              return mask_sb

        # ---------------- phase 2: scores + pv (pools per rep) ----------
        pools2 = {}

        def open_phase2_pools(rep):
            st = contextlib.ExitStack()
            pools2["skew"] = st.enter_context(
                tc.tile_pool(name=f"skew{rep}", bufs=2))
            pools2["pt"] = st.enter_context(
                tc.tile_pool(name=f"pt{rep}", bufs=8))
            pools2["misc"] = st.enter_context(
                tc.tile_pool(name=f"misc{rep}", bufs=2))
            pools2["spsum"] = st.enter_context(
                tc.tile_pool(name=f"spsum{rep}", bufs=2, space="PSUM"))
            return st

        def emit_skew_reads(rep, h):
            """skq transposed into [key, query]; skk natural [key, query].

            winq is [l, 2048] with col w' = 1023 - l + r, so
            addr(r, l) = 2047*l + r + 1023: partitions r (stride 1), one
            merged free dim l (stride 2047) — a legal 2-dim DMA AP."""
            skq_t = pools2["skew"].tile([128, 8, 1024], wdt, tag="skq",
                                        name=f"skq_{h}")
            for rb in range(8):
                nc.scalar.dma_start(
                    out=skq_t[:, rb, :],
                    in_=bass.AP(tensor=winq[rep][h], offset=1023 + rb * 128,
                                ap=[[1, 128], [WQF - 1, 1024]]))
            skk_t = pools2["skew"].tile([128, 8, 1024], wdt, tag="skk",
                                        name=f"skk_{h}")
            for hf in range(2):
                nc.scalar.dma_start(
                    out=skk_t[:, hf * 4:(hf + 1) * 4, :],
                    in_=bass.AP(tensor=wink[rep][h],
                                offset=127 + hf * 4 * 128 * WIN,
                                ap=[[WIN - 1, 128], [128 * WIN, 4], [1, 1024]]))
            # skk_t += skq_t  (same [key, (rb, l)] layout)
            nc.vector.tensor_add(
                out=skk_t.rearrange("p a b -> p (a b)"),
                in0=skk_t.rearrange("p a b -> p (a b)"),
                in1=skq_t.rearrange("p a b -> p (a b)"))
            return skk_t

        def emit_scores_pv(rep, pair, sub, sksum, mask_sb):
            base = 64 * sub
            h = 2 * pair + sub
            pts = []
            for rb in range(8):
                pt = pools2["pt"].tile([128, 1024], bf16, tag="pt",
                                       name=f"pt_{h}_{rb}")
                for lhalf in range(2):
                    sl = slice(lhalf * 512, (lhalf + 1) * 512)
                    ps_s = pools2["spsum"].tile([128, 512], f32, tag="sc",
                                      name=f"s_{h}_{rb}_{lhalf}")
                    nc.tensor.matmul(
                        ps_s,
                        lhsT=kT_sb[base:base + 64, pair, rb * 128:(rb + 1) * 128],
                        rhs=qT_sb[base:base + 64, pair, sl],
                        start=True, stop=True)
                    # ps_s = (ps_s + mask[key]) + skew_sum   (in place, PSUM)
                    nc.vector.scalar_tensor_tensor(
                        out=ps_s, in0=ps_s, scalar=mask_sb[:, rb:rb + 1],
                        in1=sksum[:, rb, sl], op0=OP.add, op1=OP.add)
                    nc.scalar.activation(out=pt[:, sl], in_=ps_s, func=AF.Exp)
                pts.append(pt)

            ctxT_ps = wpsum.tile([65, 1024], f32, tag="win", name=f"cT_{h}")
            for rc in range(8):
                for half in range(2):
                    sl = slice(half * 512, (half + 1) * 512)
                    nc.tensor.matmul(
                        ctxT_ps[:, sl],
                        lhsT=v_sb[:, rc, h, :],
                        rhs=pts[rc][:, sl],
                        start=(rc == 0), stop=(rc == 7))
            ctxT_bf = pools2["misc"].tile([65, 1024], bf16, tag="ctxT",
                                          name=f"cTs_{h}")
            nc.scalar.activation(out=ctxT_bf, in_=ctxT_ps, func=AF.Copy)
            nc.sync.dma_start(
                out=bass.AP(tensor=out_d, offset=(rep * HPC + h) * 65 * 1024,
                            ap=[[1024, 65], [1, 1024]]),
                in_=ctxT_bf)

        for rep in range(NREP):
            mask_sb = emit_phase1(rep)
            p2 = open_phase2_pools(rep)
            for pair in range(4):
                sk0 = emit_skew_reads(rep, 2 * pair)
                if pair + 1 < 4:
                    emit_windows(rep, pair + 1)
                sk1 = emit_skew_reads(rep, 2 * pair + 1)
                emit_scores_pv(rep, pair, 0, sk0, mask_sb)
                emit_scores_pv(rep, pair, 1, sk1, mask_sb)
            p2.close()

    nc.compile()
    return nc


def get_nc():
    if "nc" not in _CACHE:
        _CACHE["nc"] = _build()
    return _CACHE["nc"]


def make_in_maps(hidden_states, attention_mask, Wq, bq, Wk, bk, Wv, bv, dist_emb):
    bf = ml_dtypes.bfloat16
    f = np.float32
    hidden_states = np.asarray(hidden_states, f)
    dist_emb = np.asarray(dist_emb, f)

    b_ = np.empty((NBLOB,), bf)
    b_[HS0:WQ0] = hidden_states.transpose(0, 2, 1).astype(bf).ravel()
    b_[WQ0:WK0] = np.asarray(Wq, f).T.astype(bf).ravel()
    b_[WK0:WV0] = np.asarray(Wk, f).T.astype(bf).ravel()
    b_[WV0:EM0] = np.asarray(Wv, f).T.astype(bf).ravel()
    emt = np.zeros((64, 2048), f)
    emt[:, :2047] = dist_emb.T
    b_[EM0:EMR0] = emt.astype(bf).ravel()
    # q-side (reversed) table pre-scaled by 1/8: winq = q @ emr/8
    emr = np.zeros((64, 2048), f)
    emr[:, :2047] = dist_emb[::-1].T * 0.125
    b_[EMR0:BQ0] = emr.astype(bf).ravel()
    b_[BQ0:BK0] = np.asarray(bq, f).astype(bf)
    # k pre-scaled by 1/8 on device; bias must match
    b_[BK0:MK0] = (np.asarray(bk, f) * 0.125).astype(bf)
    b_[MK0:NBLOB] = np.asarray(attention_mask, f).reshape(B, S).astype(bf).ravel()
    return [{"blob": b_}]


def assemble(results, bv):
    ctx = np.asarray(results[0]["ctx"]).reshape(NREP, HPC, 65, 1024)
    ctx = ctx.astype(np.float32)
    # rows 0..63 = unnormalized ctx^T, row 64 = softmax denominator
    ctx_t = ctx[:, :, 0:64, :] / ctx[:, :, 64:65, :]     # [rep, h, d, l]
    ctx_t = ctx_t.transpose(0, 3, 1, 2).reshape(NREP, S, HPC * D)
    out = np.empty((B, S, E), np.float32)
    for rep in range(NREP):
        b, g = divmod(rep, 2)
        out[b, :, EOUT * g:EOUT * (g + 1)] = ctx_t[rep]
    out += np.asarray(bv, np.float32)[None, None, :]
    return out


def kernel(hidden_states, attention_mask, Wq, bq, Wk, bk, Wv, bv, dist_emb,
           trace=False):
    global LAST_RESULTS
    from concourse import bass_utils
    nc = get_nc()
    in_maps = make_in_maps(hidden_states, attention_mask, Wq, bq, Wk, bk, Wv,
                           bv, dist_emb)
    res = bass_utils.run_bass_kernel_spmd(nc, in_maps, core_ids=[0],
                                          trace=trace)
    LAST_RESULTS = res
    return assemble(res.results, bv)


# revision 11
# speedup vs baseline: 1.5793x; 1.2300x over previous
"""
Trainium2 Bass kernel for nn_BertSelfAttention_1580547972513 — v5.

Single core, one packed bf16 input blob, bf16 output (v4), plus a
restructure driven by the measured device cost model (~100us per matmul
instruction regardless of shape; DVE ops ~15us; strided/transposing DMA
reads ~free):

  - q-side skew windows are read back from DRAM with a TRANSPOSED strided
    AP directly into [key, query] orientation — removes the 4 identity-
    transpose matmuls per score chunk (512 matmuls/rep).
  - both skews are summed with one DVE tensor_add per head and applied to
    scores with one fused scalar_tensor_tensor (+mask) per chunk, in
    PSUM — removes the per-chunk identity add-matmul (128 matmuls/rep).
  - kT and the q-window table are pre-scaled by 1/8 (host side for the
    table, fused into the k-projection eviction) so no separate score
    scaling is needed.
  - context is shipped unnormalized+transposed as [65,1024] per head
    (64 dims + denominator row); the softmax divide and [d,l]->[l,d]
    transpose happen on the host — removes 64 transpose matmuls and 128
    DVE ops per rep.

matmuls/rep: 192 proj + 384 windows + 128 scores + 128 pv = 832
(v4 had 1536).
"""

import numpy as np
import ml_dtypes

B, S, E, H, D, MAXP = 4, 1024, 1024, 16, 64, 1024
HPC = 8          # heads per config
EOUT = 512       # projection out-cols per config
WIN = 1152
NREP = 8         # (b, g) configs, all on core 0

# ---- blob element offsets (bf16) ----
HS0 = 0                       # [B][E][S]
WQ0 = HS0 + B * E * S         # [E][H*D]
WK0 = WQ0 + E * H * D
WV0 = WK0 + E * H * D
EM0 = WV0 + E * H * D         # [64][2048]
EMR0 = EM0 + 64 * 2048
BQ0 = EMR0 + 64 * 2048        # [H*D]
BK0 = BQ0 + H * D
MK0 = BK0 + H * D             # [B][S]
NBLOB = MK0 + B * S

_CACHE = {}
LAST_RESULTS = None


def _build():
    import concourse.bacc as bacc
    import concourse.bass as bass
    import concourse.mybir as mybir
    import concourse.tile as tile
    from concourse.masks import make_identity
    from contextlib import ExitStack
    import contextlib

    f32 = mybir.dt.float32
    f32r = mybir.dt.float32r
    bf16 = mybir.dt.bfloat16
    wdt = bf16
    AF = mybir.ActivationFunctionType
    OP = mybir.AluOpType

    nc = bacc.Bacc("TRN2", target_bir_lowering=False, debug=False)

    blob = nc.dram_tensor("blob", [NBLOB], bf16, kind="ExternalInput")
    out_d = nc.dram_tensor("ctx", [NREP * HPC * 65 * 1024], bf16,
                           kind="ExternalOutput")

    # q-windows stored UNBANDED [l, 2048] so the transposed skew read
    # merges (lb, p) into one stride-2047 dim (3-dim DMA AP limit).
    WQF = 2048
    winq = [[nc.dram_tensor(f"winq{r}_{h}", [S * WQF], wdt, kind="Internal")
             for h in range(HPC)] for r in range(NREP)]
    wink = [[nc.dram_tensor(f"wink{r}_{h}", [S * WIN], wdt, kind="Internal")
             for h in range(HPC)] for r in range(NREP)]

    def bap(offset, dims):
        return bass.AP(tensor=blob, offset=offset, ap=dims)

    with tile.TileContext(nc) as tc, ExitStack() as top:
        const = top.enter_context(tc.tile_pool(name="const", bufs=1))

        # ---------------- constants ----------------
        em_sb = const.tile([128, 2048], f32r)
        emr_sb = const.tile([128, 2048], f32r)
        with tc.tile_pool(name="emtmp", bufs=1) as emtmp:
            em_bf = emtmp.tile([128, 2048], bf16)
            nc.sync.dma_start(out=em_bf[0:64, :],
                              in_=bap(EM0, [[2048, 64], [1, 2048]]))
            nc.sync.dma_start(out=em_bf[64:128, :],
                              in_=bap(EM0, [[2048, 64], [1, 2048]]))
            nc.vector.tensor_copy(out=em_sb, in_=em_bf)
            emr_bf = emtmp.tile([128, 2048], bf16)
            nc.sync.dma_start(out=emr_bf[0:64, :],
                              in_=bap(EMR0, [[2048, 64], [1, 2048]]))
            nc.sync.dma_start(out=emr_bf[64:128, :],
                              in_=bap(EMR0, [[2048, 64], [1, 2048]]))
            nc.vector.tensor_copy(out=emr_sb, in_=emr_bf)

        qT_sb = const.tile([128, 4, 1024], f32r)
        kT_sb = const.tile([128, 4, 1024], f32r)
        v_sb = const.tile([128, 8, HPC, 65], bf16)
        nc.vector.memset(v_sb[:, :, :, 64:65], 1.0)

        persist = top.enter_context(tc.tile_pool(name="persist", bufs=2))
        stage_pool = top.enter_context(tc.tile_pool(name="stage", bufs=2))
        wpsum = top.enter_context(tc.tile_pool(name="wpsum", bufs=2, space="PSUM"))

        def emit_windows(rep, pair):
            """Banded window matmuls for both sides, two heads interleaved
            (base partitions 0/64).  The k-side is stored banded
            ([l, w-w0(lb)] rows of WIN).  The q-side computes the same band
            but SCATTERS it into an unbanded [l, 2048] row layout — the
            per-block offset w0 = 896-128*lb folds into the li-stride
            (128*WQF - 128) — so the skew read-back can be a legal 2-dim
            transposed AP."""
            for side, (src_sb, tab_sb) in enumerate(
                    ((qT_sb, emr_sb), (kT_sb, em_sb))):
                for half in range(2):
                    stages = []
                    for sub in range(2):
                        stages.append(stage_pool.tile(
                            [128, 4, WIN], wdt, tag="stage",
                            name=f"st_{side}_{2 * pair + sub}_{half}"))
                    for li in range(4):
                        lb = half * 4 + li
                        w0 = 896 - 128 * lb
                        pss = [wpsum.tile([128, 3, 512], f32, tag="win",
                                          name=f"w_{side}_{2 * pair + sub}_{lb}")
                               for sub in range(2)]
                        for c in range(3):
                            for sub in range(2):
                                base = 64 * sub
                                nc.tensor.matmul(
                                    pss[sub][:, c, 0:384],
                                    lhsT=src_sb[base:base + 64, pair,
                                                lb * 128:(lb + 1) * 128],
                                    rhs=tab_sb[base:base + 64,
                                               w0 + c * 384: w0 + (c + 1) * 384],
                                    start=True, stop=True)
                        for sub in range(2):
                            dst3 = stages[sub][:, li, :].rearrange(
                                "p (a b) -> p a b", b=384)
                            if (lb + sub) % 2 == 0:
                                nc.vector.tensor_copy(out=dst3,
                                                      in_=pss[sub][:, :, 0:384])
                            else:
                                nc.scalar.activation(out=dst3,
                                                     in_=pss[sub][:, :, 0:384],
                                                     func=AF.Copy)
                    for sub in range(2):
                        h = 2 * pair + sub
                        if side == 0:
                            # unbanded scatter: addr = l*WQF + w0(lb) + wloc
                            out_ap = bass.AP(
                                tensor=winq[rep][h],
                                offset=half * 4 * 128 * WQF + 896 - 512 * half,
                                ap=[[WQF, 128], [128 * WQF - 128, 4], [1, WIN]])
                        else:
                            out_ap = bass.AP(
                                tensor=wink[rep][h], offset=half * 4 * 128 * WIN,
                                ap=[[WIN, 128], [128 * WIN, 4], [1, WIN]])
                        nc.sync.dma_start(out=out_ap, in_=stages[sub])

        # ---------------- phase 1: projections interleaved with windows ----
        def emit_phase1(rep):
          b, g = divmod(rep, 2)
          with tc.tile_pool(name=f"hs{rep}", bufs=1) as hspool, \
               tc.tile_pool(name=f"ppsum{rep}", bufs=2, space="PSUM") as ppsum:
              bq_bf = hspool.tile([128, 4], bf16)
              nc.scalar.dma_start(out=bq_bf,
                                  in_=bap(BQ0 + g * 512, [[1, 128], [128, 4]]))
              bq_sb = hspool.tile([128, 4], f32)
              nc.vector.tensor_copy(out=bq_sb, in_=bq_bf)
              bk_bf = hspool.tile([128, 4], bf16)
              nc.scalar.dma_start(out=bk_bf,
                                  in_=bap(BK0 + g * 512, [[1, 128], [128, 4]]))
              bk_sb = hspool.tile([128, 4], f32)
              nc.vector.tensor_copy(out=bk_sb, in_=bk_bf)
              mk_bf = hspool.tile([128, 8], bf16)
              nc.scalar.dma_start(out=mk_bf,
                                  in_=bap(MK0 + b * S, [[1, 128], [128, 8]]))
              # lives into phase 2 (score bias) — long-lived pool
              mask_sb = persist.tile([128, 8], f32, tag="mask",
                                     name=f"mask_{rep}")
              nc.vector.tensor_copy(out=mask_sb, in_=mk_bf)

              hs_sb = hspool.tile([128, 8, 1024], bf16)
              wq_sb = hspool.tile([128, 8, EOUT], bf16)
              wk_sb = hspool.tile([128, 8, EOUT], bf16)
              wv_sb = hspool.tile([128, 8, EOUT], bf16)
              hsr = bap(HS0 + b * E * S, [[S, 128], [128 * S, 8], [1, S]])
              wqr = bap(WQ0 + g * 512, [[1024, 128], [128 * 1024, 8], [1, 512]])
              wkr = bap(WK0 + g * 512, [[1024, 128], [128 * 1024, 8], [1, 512]])
              wvr = bap(WV0 + g * 512, [[1024, 128], [128 * 1024, 8], [1, 512]])
              # interleave so (hs0, wq0) land first
              for cc in range(8):
                  nc.scalar.dma_start(out=hs_sb[:, cc:cc + 1, :],
                                      in_=hsr[:, cc:cc + 1, :])
                  nc.scalar.dma_start(out=wq_sb[:, cc:cc + 1, :],
                                      in_=wqr[:, cc:cc + 1, :])
                  nc.scalar.dma_start(out=wk_sb[:, cc:cc + 1, :],
                                      in_=wkr[:, cc:cc + 1, :])
              nc.scalar.dma_start(out=wv_sb, in_=wvr)

              def proj_qk(w_sb, dst, b_sb, j, prescale):
                  for half in range(2):
                      ps = ppsum.tile([128, 512], f32, tag="proj",
                                      name=f"ps_{j}_{half}")
                      for e in range(8):
                          nc.tensor.matmul(
                              ps,
                              lhsT=w_sb[:, e, j * 128:(j + 1) * 128],
                              rhs=hs_sb[:, e, half * 512:(half + 1) * 512],
                              start=(e == 0), stop=(e == 7))
                      dst_sl = dst[:, j, half * 512:(half + 1) * 512]
                      if prescale is None:
                          nc.vector.tensor_scalar_add(
                              out=dst_sl, in0=ps, scalar1=b_sb[:, j:j + 1])
                      else:
                          # dst = ps*prescale + bias  (bias pre-scaled on host)
                          nc.vector.tensor_scalar(
                              out=dst_sl, in0=ps, scalar1=prescale,
                              scalar2=b_sb[:, j:j + 1],
                              op0=OP.mult, op1=OP.add)

              proj_qk(wq_sb, qT_sb, bq_sb, 0, None)
              proj_qk(wk_sb, kT_sb, bk_sb, 0, 0.125)
              emit_windows(rep, 0)
              for p in range(1, 4):
                  proj_qk(wq_sb, qT_sb, bq_sb, p, None)
                  proj_qk(wk_sb, kT_sb, bk_sb, p, 0.125)

              for t in range(8):
                  psv = ppsum.tile([128, 512], f32, tag="proj", name=f"psv_{t}")
                  for e in range(8):
                      nc.tensor.matmul(
                          psv,
                          lhsT=hs_sb[:, e, t * 128:(t + 1) * 128],
                          rhs=wv_sb[:, e, :],
                          start=(e == 0), stop=(e == 7))
                  nc.vector.tensor_copy(
                      out=v_sb[:, t, :, 0:64],
                      in_=psv.rearrange("p (h d) -> p h d", d=64))
              return mask_sb

        # ---------------- phase 2: scores + pv (pools per rep) ----------
        pools2 = {}

        def open_phase2_pools(rep):
            st = contextlib.ExitStack()
            pools2["skew"] = st.enter_context(
                tc.tile_pool(name=f"skew{rep}", bufs=2))
            pools2["pt"] = st.enter_context(
                tc.tile_pool(name=f"pt{rep}", bufs=8))
            pools2["misc"] = st.enter_context(
                tc.tile_pool(name=f"misc{rep}", bufs=2))
            pools2["spsum"] = st.enter_context(
                tc.tile_pool(name=f"spsum{rep}", bufs=2, space="PSUM"))
            return st

        def emit_skew_reads(rep, h):
            """skq transposed into [key, query]; skk natural [key, query].

            winq is [l, 2048] with col w' = 1023 - l + r, so
            addr(r, l) = 2047*l + r + 1023: partitions r (stride 1), one
            merged free dim l (stride 2047) — a legal 2-dim DMA AP."""
            skq_t = pools2["skew"].tile([128, 8, 1024], wdt, tag="skq",
                                        name=f"skq_{h}")
            for rb in range(8):
                nc.scalar.dma_start(
                    out=skq_t[:, rb, :],
                    in_=bass.AP(tensor=winq[rep][h], offset=1023 + rb * 128,
                                ap=[[1, 128], [WQF - 1, 1024]]))
            skk_t = pools2["skew"].tile([128, 8, 1024], wdt, tag="skk",
                                        name=f"skk_{h}")
            for hf in range(2):
                nc.scalar.dma_start(
                    out=skk_t[:, hf * 4:(hf + 1) * 4, :],
                    in_=bass.AP(tensor=wink[rep][h],
                                offset=127 + hf * 4 * 128 * WIN,
                                ap=[[WIN - 1, 128], [128 * WIN, 4], [1, 1024]]))
            # skk_t += skq_t  (same [key, (rb, l)] layout)
            nc.vector.tensor_add(
                out=skk_t.rearrange("p a b -> p (a b)"),
                in0=skk_t.rearrange("p a b -> p (a b)"),
                in1=skq_t.rearrange("p a b -> p (a b)"))
            return skk_t

        def emit_scores_pv(rep, pair, sub, sksum, mask_sb):
            base = 64 * sub
            h = 2 * pair + sub
            pts = []
            for rb in range(8):
                pt = pools2["pt"].tile([128, 1024], bf16, tag="pt",
                                       name=f"pt_{h}_{rb}")
                for lhalf in range(2):
                    sl = slice(lhalf * 512, (lhalf + 1) * 512)
                    ps_s = pools2["spsum"].tile([128, 512], f32, tag="sc",
                                      name=f"s_{h}_{rb}_{lhalf}")
                    nc.tensor.matmul(
                        ps_s,
                        lhsT=kT_sb[base:base + 64, pair, rb * 128:(rb + 1) * 128],
                        rhs=qT_sb[base:base + 64, pair, sl],
                        start=True, stop=True)
                    # ps_s = (ps_s + mask[key]) + skew_sum   (in place, PSUM)
                    nc.vector.scalar_tensor_tensor(
                        out=ps_s, in0=ps_s, scalar=mask_sb[:, rb:rb + 1],
                        in1=sksum[:, rb, sl], op0=OP.add, op1=OP.add)
                    nc.scalar.activation(out=pt[:, sl], in_=ps_s, func=AF.Exp)
                pts.append(pt)

            ctxT_ps = wpsum.tile([65, 1024], f32, tag="win", name=f"cT_{h}")
            for rc in range(8):
                for half in range(2):
                    sl = slice(half * 512, (half + 1) * 512)
                    nc.tensor.matmul(
                        ctxT_ps[:, sl],
                        lhsT=v_sb[:, rc, h, :],
                        rhs=pts[rc][:, sl],
                        start=(rc == 0), stop=(rc == 7))
            ctxT_bf = pools2["misc"].tile([65, 1024], bf16, tag="ctxT",
                                          name=f"cTs_{h}")
            nc.scalar.activation(out=ctxT_bf, in_=ctxT_ps, func=AF.Copy)
            nc.sync.dma_start(
                out=bass.AP(tensor=out_d, offset=(rep * HPC + h) * 65 * 1024,
                            ap=[[1024, 65], [1, 1024]]),
                in_=ctxT_bf)

        for rep in range(NREP):
            mask_sb = emit_phase1(rep)
            p2 = open_phase2_pools(rep)
            for pair in range(4):
                sk0 = emit_skew_reads(rep, 2 * pair)
                if pair + 1 < 4:
                    emit_windows(rep, pair + 1)
                sk1 = emit_skew_reads(rep, 2 * pair + 1)
                emit_scores_pv(rep, pair, 0, sk0, mask_sb)
                emit_scores_pv(rep, pair, 1, sk1, mask_sb)
            p2.close()

    nc.compile()
    return nc


def get_nc():
    if "nc" not in _CACHE:
        _CACHE["nc"] = _build()
    return _CACHE["nc"]


def make_in_maps(hidden_states, attention_mask, Wq, bq, Wk, bk, Wv, bv, dist_emb):
    bf = ml_dtypes.bfloat16
    f = np.float32
    hidden_states = np.asarray(hidden_states, f)
    dist_emb = np.asarray(dist_emb, f)

    b_ = np.empty((NBLOB,), bf)
    b_[HS0:WQ0] = hidden_states.transpose(0, 2, 1).astype(bf).ravel()
    b_[WQ0:WK0] = np.asarray(Wq, f).T.astype(bf).ravel()
    b_[WK0:WV0] = np.asarray(Wk, f).T.astype(bf).ravel()
    b_[WV0:EM0] = np.asarray(Wv, f).T.astype(bf).ravel()
    emt = np.zeros((64, 2048), f)
    emt[:, :2047] = dist_emb.T
    b_[EM0:EMR0] = emt.astype(bf).ravel()
    # q-side (reversed) table pre-scaled by 1/8: winq = q @ emr/8
    emr = np.zeros((64, 2048), f)
    emr[:, :2047] = dist_emb[::-1].T * 0.125
    b_[EMR0:BQ0] = emr.astype(bf).ravel()
    b_[BQ0:BK0] = np.asarray(bq, f).astype(bf)
    # k pre-scaled by 1/8 on device; bias must match
    b_[BK0:MK0] = (np.asarray(bk, f) * 0.125).astype(bf)
    b_[MK0:NBLOB] = np.asarray(attention_mask, f).reshape(B, S).astype(bf).ravel()
    return [{"blob": b_}]


def assemble(results, bv):
    ctx = np.asarray(results[0]["ctx"]).reshape(NREP, HPC, 65, 1024)
    ctx = ctx.astype(np.float32)
    # rows 0..63 = unnormalized ctx^T, row 64 = softmax denominator
    ctx_t = ctx[:, :, 0:64, :] / ctx[:, :, 64:65, :]     # [rep, h, d, l]
    ctx_t = ctx_t.transpose(0, 3, 1, 2).reshape(NREP, S, HPC * D)
    out = np.empty((B, S, E), np.float32)
    for rep in range(NREP):
        b, g = divmod(rep, 2)
        out[b, :, EOUT * g:EOUT * (g + 1)] = ctx_t[rep]
    out += np.asarray(bv, np.float32)[None, None, :]
    return out


def kernel(hidden_states, attention_mask, Wq, bq, Wk, bk, Wv, bv, dist_emb,
           trace=False):
    global LAST_RESULTS
    from concourse import bass_utils
    nc = get_nc()
    in_maps = make_in_maps(hidden_states, attention_mask, Wq, bq, Wk, bk, Wv,
                           bv, dist_emb)
    res = bass_utils.run_bass_kernel_spmd(nc, in_maps, core_ids=[0],
                                          trace=trace)
    LAST_RESULTS = res
    return assemble(res.results, bv)
